# revision 56
# baseline (speedup 1.0000x reference)
"""Distributed Trainium2 kernel for AM-normfree-softmax + MHE inter-class loss.

loss = CE(S*(emb @ normalize(W).T - M*onehot(y)), y)
       + sum_{i, j != y_i} 1/||w_hat_{y_i} - w_hat_j||^2 / (B*(C-1))

Strategy (classifier/tensor parallel): shard the class dim C across 8 cores.
Each core holds its W-shard TRANSPOSED (D, C/8) as the moving matmul operand,
normalizes it on-device (square -> PE ones-matmul column sum-of-squares ->
sqrt -> reciprocal -> PE broadcast -> multiply), and computes
  [emb; W[y]] @ W_hat_shard.T          (stationary = emb.T | W[y].T, bf16)
in one fused matmul per (M-tile, N-chunk) with an extra rank-<=128 K-tile
(U @ V) that subtracts BIG at (i, y_i) so the MHE diagonal self-distance is
suppressed inside the matmul itself (no masking pass needed).

CE: per-row online two-level logsumexp: per-core row-max from the first
chunk fixes a per-row exp bias; ACT Exp with accum_out reduces each chunk
on the fly.  MHE: ACT affine (2 - 2*inv_i*g) from PSUM + DVE reciprocal +
row-reduce.  One AllGather of (bias, sumexp, inter-partial) merges everything;
each core redundantly computes the final scalar.

Only the final (1,1) from core 0 is returned.
"""

import math
from functools import lru_cache

import ml_dtypes
import numpy as np

import concourse.bass as bass
import concourse.bass_isa as bass_isa
import concourse.tile as tile
from concourse import bacc, mybir
from concourse.bass_utils import run_bass_kernel_spmd

# Pin ACT to the one table set containing every function we use
# (exp, ln, copy, identity, square).  The default per-function set picker
# alternates exp_and_others <-> natural_log, reloading tables (~1.5us + drain)
# dozens of times per kernel.  Emptying the other sets' overlapping entries
# (list order and hence act_func_set_id stay intact) forces a single load.
_ACT_KEEP = "natural_log_exp_and_others"
_orig_gat = bacc.get_activation_tables


def _pinned_gat(arch):
    tabs = _orig_gat(arch)
    shared = tabs[_ACT_KEEP]
    return {name: (fns if name == _ACT_KEEP else fns - shared)
            for name, fns in tabs.items()}


bacc.get_activation_tables = _pinned_gat

# ---- custom fused DVE op: accum += sum_k approx(1/(x_k - s0)) ----
# Collapses the MHE epilogue (affine + reciprocal + row-reduce, one ACT op +
# two DVE ops per tile) into a single DVE instruction:
#   sum_j 1/(2 - 2*inv_i*g_ij) == (-nrm_i/2) * sum_j 1/(g_ij - nrm_i)
# so per tile we only need the shifted reciprocal-sum; the (-nrm_i/2) row
# scale is applied once per row at merge time.  BITWISE_NOT exponent-flip
# seed + one Newton pass (naive -4/17 seed consts): ~0.3% systematic error
# on 1/d, i.e. ~1e-5 of the total loss.  6 ALU slices + accumulate.
from operator import add as _op_add  # noqa: E402

import concourse.dve_ops as _dve_ops  # noqa: E402
from concourse.dve_spec import (  # noqa: E402
    AluOp as _DAluOp,
    Bin as _DBin,
    C0 as _DC0,
    C1 as _DC1,
    C2 as _DC2,
    Spec as _DSpec,
    Src0 as _DSrc0,
    Zero as _DZero,
    _has_src1 as _dve_has_src1,
    lower as _dve_lower,
)
from concourse.dve_uop import DveOpSpec as _DveOpSpec  # noqa: E402

_SRA_NAME = "SHIFTED_RECIP_ACC_ANT"


def _sra_reference(in0, in1, s0, s1, imm2):
    x = np.ascontiguousarray(in0.astype(np.float32) - s0)
    nx = (~x.view(np.int32)).view(np.float32)
    y0 = nx * np.float32(s1)
    y1 = (y0 * (np.float32(imm2) - x * y0)).astype(np.float32)
    return y1, y1.reshape(y1.shape[0], -1).sum(axis=-1, keepdims=True)


def _register_sra():
    for op in _dve_ops.OPS:
        if op.name == _SRA_NAME:
            return op
    d = _DSrc0 - _DC0
    nd = _DBin(_DAluOp.BITWISE_NOT, d, d)
    y0 = nd * _DC1
    y1 = y0 * (_DC2 - d * y0)
    spec = _DSpec(body=y1, accum=_op_add, accum_init=_DZero,
                  reference=_sra_reference)
    row = max(_dve_ops._SUB_OPCODE_FOR_NAME.values()) + 1
    assert row < 0x20
    _dve_ops._SUB_OPCODE_FOR_NAME[_SRA_NAME] = row
    shas = {}
    for ver in ("v3", "v4"):
        tmp = _DveOpSpec(name=_SRA_NAME, opcode=row,
                         uops=_dve_lower(spec, ver=ver),
                         rd1_en=_dve_has_src1(spec))
        shas[ver] = tmp.sha(ver)
    op = _dve_ops.DveOp(_SRA_NAME, spec, subdim=False, uops_sha=shas)
    _dve_ops.OPS.append(op)
    _dve_ops.CUSTOM_DVE_SPECS[_SRA_NAME] = spec
    return op


_SRA_OP = _register_sra()
_SRA_SEED = -4.0 / 17.0

F32 = mybir.dt.float32
BF16 = mybir.dt.bfloat16
AX = mybir.AxisListType
ALU = mybir.AluOpType
ACTF = mybir.ActivationFunctionType

B, D, C = 512, 512, 50000
NCORES = 8
CSH = C // NCORES          # 6250 classes per core
CPAD = 6272                # 49 * 128, padded shard width
S_SCALE = 30.0
MARGIN = 0.2
LMD = 1.0
BIG = float(2 ** 24)

# N-chunks over the padded shard: the small 128-col chunk first (cheapest
# prologue: its DMA+normalize chain gates the first matmul), then 12 x 512
CHUNKS = [(6144, 128)] + [(j * 512, 512) for j in range(12)]
NCHUNK = len(CHUNKS)
KB = D // 128              # 4 contraction blocks
MT = B // 128              # 4 M-tiles per operand group (emb rows / ws rows)


def _build_graph(nst: int, stage: str = "full"):
    """Build the SPMD Bass graph. nst = number of 128-slot U/V K-tiles.

    stage: debug knob — "prep", "norm", "mm", or "full".
    """
    nc = bacc.Bacc("TRN2", target_bir_lowering=False, debug=False,
                   num_devices=NCORES)

    wt = nc.declare_dram_parameter("wt", [D, CPAD], BF16, isOutput=False)
    embT = nc.declare_dram_parameter("embt", [D, B], BF16, isOutput=False)
    wsT = nc.declare_dram_parameter("wst", [D, B], BF16, isOutput=False)
    emb = nc.declare_dram_parameter("emb", [B, D], F32, isOutput=False)
    ws = nc.declare_dram_parameter("ws", [B, D], F32, isOutput=False)
    u_p = nc.declare_dram_parameter("u", [nst * 128, B], BF16, isOutput=False)
    v_p = nc.declare_dram_parameter("v", [nst * 128, CPAD], BF16, isOutput=False)
    out_p = nc.declare_dram_parameter("out", [1, 1], F32, isOutput=True)

    cc_in = nc.dram_tensor("cc_in", [128, 9], F32)
    cc_out = nc.dram_tensor("cc_out", [NCORES, 128, 9], F32, addr_space="Shared")

    with tile.TileContext(nc) as tc:
        with (
            tc.tile_pool(name="consts", bufs=1) as consts,
            tc.tile_pool(name="stat", bufs=1) as statp,
            tc.tile_pool(name="persist", bufs=1) as pers,
            tc.tile_pool(name="wstage", bufs=6) as wstage_p,
            tc.tile_pool(name="sq", bufs=6) as sq_p,
            tc.tile_pool(name="nrm", bufs=2) as nrm_p,
            tc.tile_pool(name="escr", bufs=3) as escr_p,
            tc.tile_pool(name="rscr", bufs=3) as rscr_p,
            tc.tile_pool(name="mrg", bufs=1) as mrg_p,
            tc.tile_pool(name="ps_main", bufs=4, space="PSUM") as ps_main,
            tc.tile_pool(name="ps_ssq", bufs=2, space="PSUM") as ps_ssq,
            tc.tile_pool(name="ps_inv", bufs=2, space="PSUM") as ps_inv,
        ):
            # ---- constants ----
            ones_col = consts.tile([128, 1], BF16)     # sumsq lhsT (K=128,M=1)
            nc.vector.memset(ones_col, 1.0)
            ones_row = consts.tile([1, 128], BF16)     # bcast lhsT (K=1,M=128)
            nc.vector.memset(ones_row, 1.0)
            eps_t = consts.tile([1, 1], F32)           # sqrt bias for pad cols
            nc.vector.memset(eps_t, 1e-30)
            # dummy activation traced first: pulls the one-time ACT table load
            # (~2.7us) off the first chunk's critical path
            warm_t = consts.tile([1, 1], F32)
            nc.scalar.activation(warm_t, eps_t, ACTF.Square)

            # ---- chunk-0 W DMA + square first: shortens the prologue (the
            # first matmuls depend on this chain, not on the stationaries)
            wst0 = None
            if stage != "prep":
                c0_0, nco_0 = CHUNKS[0]
                wst0 = wstage_p.tile([128, KB, 512], BF16, tag="wstage")
                sq0 = sq_p.tile([128, KB, 512], BF16, tag="sq")
                for kb in range(KB):
                    nc.sync.dma_start(
                        out=wst0[:, kb, :nco_0],
                        in_=wt[kb * 128:(kb + 1) * 128, c0_0:c0_0 + nco_0])
                    nc.scalar.activation(sq0[:, kb, :nco_0],
                                         wst0[:, kb, :nco_0], ACTF.Square)

            # ---- stationary operands (embT/wsT on the sync queue right after
            # chunk 0; bulky V/emb/ws on the gpsimd queue in parallel) ----
            embT_sb = statp.tile([128, KB, B], BF16)
            wsT_sb = statp.tile([128, KB, B], BF16)
            for kb in range(KB):
                nc.sync.dma_start(out=embT_sb[:, kb, :],
                                  in_=embT[kb * 128:(kb + 1) * 128, :])
                nc.sync.dma_start(out=wsT_sb[:, kb, :],
                                  in_=wsT[kb * 128:(kb + 1) * 128, :])
            u_sb = statp.tile([128, nst, B], BF16)
            for st in range(nst):
                nc.gpsimd.dma_start(out=u_sb[:, st, :],
                                    in_=u_p[st * 128:(st + 1) * 128, :])
            v_sb = statp.tile([128, nst, CPAD], BF16)
            for st in range(nst):
                nc.gpsimd.dma_start(out=v_sb[:, st, :],
                                    in_=v_p[st * 128:(st + 1) * 128, :])

            # natural-layout emb/ws for target-logit extraction
            emb_sb = statp.tile([128, MT, D], F32)
            ws_sb = statp.tile([128, MT, D], F32)
            for m in range(MT):
                nc.gpsimd.dma_start(out=emb_sb[:, m, :],
                                    in_=emb[m * 128:(m + 1) * 128, :])
                nc.gpsimd.dma_start(out=ws_sb[:, m, :],
                                    in_=ws[m * 128:(m + 1) * 128, :])

            # ---- ws row norms + target logits (replicated on all cores) ----
            ssq_ws = pers.tile([128, MT], F32)
            dot_t = pers.tile([128, MT], F32)
            ttr_scr = pers.tile([128, D], F32)
            for m in range(MT):
                nc.vector.tensor_mul(ttr_scr, ws_sb[:, m, :], ws_sb[:, m, :])
                nc.vector.reduce_sum(ssq_ws[:, m:m + 1], ttr_scr, axis=AX.X)
            for m in range(MT):
                nc.vector.tensor_mul(ttr_scr, emb_sb[:, m, :], ws_sb[:, m, :])
                nc.vector.reduce_sum(dot_t[:, m:m + 1], ttr_scr, axis=AX.X)
            # inv_ws = rsqrt(ssq) = exp(-0.5*ln(ssq)); Ln/Exp share one ACT
            # table set (no per-use table reloads, no slow DVE reciprocal)
            lnv_ws = pers.tile([128, MT], F32)
            nc.scalar.activation(lnv_ws, ssq_ws, ACTF.Ln)
            inv_ws = pers.tile([128, MT], F32)
            nc.scalar.activation(inv_ws, lnv_ws, ACTF.Exp, scale=-0.5)
            nrm_ws = pers.tile([128, MT], F32)     # ||w_{y_i}||, SRA shift
            nc.scalar.activation(nrm_ws, lnv_ws, ACTF.Exp, scale=0.5)
            scl_ws = pers.tile([128, MT], F32)     # -nrm/2, SRA row scale
            nc.vector.tensor_scalar_mul(scl_ws, nrm_ws, -0.5)
            # tgt = S*(inv_ws*dot - MARGIN)
            cosiy = pers.tile([128, MT], F32)
            nc.vector.tensor_mul(cosiy, dot_t, inv_ws)
            tgt_t = pers.tile([128, MT], F32)
            nc.vector.tensor_scalar(out=tgt_t, in0=cosiy,
                                    scalar1=S_SCALE, scalar2=-S_SCALE * MARGIN,
                                    op0=ALU.mult, op1=ALU.add)

            # ---- persistent accumulators ----
            wh = pers.tile([128, KB, CPAD], BF16)       # normalized W-shard.T
            bias_t = pers.tile([128, MT], F32)          # per-row exp bias
            sslots = pers.tile([128, MT, NCHUNK], F32)  # per-chunk exp sums
            islots = pers.tile([128, MT, NCHUNK], F32)  # per-chunk 1/d2 sums

            if stage == "prep":
                # touch wt so the ExternalInput isn't pruned
                wtouch = wstage_p.tile([128, 512], BF16, tag="wtouch")
                nc.sync.dma_start(out=wtouch, in_=wt[0:128, 0:512])

            # ---- main loop over N-chunks ----
            chunk_list = [] if stage == "prep" else (
                CHUNKS if stage in ("mm", "full") else CHUNKS[:2])
            # pair the 512-col chunks so each stationary load covers two
            # moving streams (halves exposed LDWEIGHTS); chunk 0 (128 cols)
            # runs alone to keep the prologue short
            groups, idx = [], 0
            while idx < len(chunk_list):
                n = 1 if idx < 4 else min(2, len(chunk_list) - idx)
                groups.append(list(range(idx, idx + n)))
                idx += n

            def normalize_chunk(j):
                c0, nco = chunk_list[j]
                if j == 0 and wst0 is not None:
                    wstage, sq = wst0, sq0
                else:
                    wstage = wstage_p.tile([128, KB, 512], BF16, tag="wstage",
                                           name=f"wstage{j}")
                    sq = sq_p.tile([128, KB, 512], BF16, tag="sq",
                                   name=f"sq{j}")
                    for kb in range(KB):
                        nc.sync.dma_start(
                            out=wstage[:, kb, :nco],
                            in_=wt[kb * 128:(kb + 1) * 128, c0:c0 + nco])
                        nc.scalar.activation(sq[:, kb, :nco],
                                             wstage[:, kb, :nco],
                                             ACTF.Square)
                ssq_ps = ps_ssq.tile([1, 512], F32, tag="ssq", name=f"ssq{j}")
                for kb in range(KB):
                    nc.tensor.matmul(ssq_ps[:, :nco], ones_col,
                                     sq[:, kb, :nco],
                                     start=(kb == 0), stop=(kb == KB - 1))
                # inv = rsqrt(ssq + eps) = exp(-0.5*ln(ssq + eps))
                nrm = nrm_p.tile([1, 512], F32, tag="nrm", name=f"nrm{j}")
                nc.scalar.activation(nrm[:, :nco], ssq_ps[:, :nco],
                                     ACTF.Ln, bias=eps_t[:, :])
                inv_bf = nrm_p.tile([1, 512], BF16, tag="invbf",
                                    name=f"invbf{j}")
                nc.scalar.activation(inv_bf[:, :nco], nrm[:, :nco],
                                     ACTF.Exp, scale=-0.5)
                invB = ps_inv.tile([128, 512], F32, tag="invB",
                                   name=f"invB{j}")
                nc.tensor.matmul(invB[:, :nco], ones_row, inv_bf[:, :nco],
                                 start=True, stop=True)
                # PSUM->SBUF bf16 copy on ACT so the normalize multiply runs
                # bf16 x bf16 at the DVE 2x rate
                invS = nrm_p.tile([128, 512], BF16, tag="invS",
                                  name=f"invS{j}")
                nc.scalar.activation(invS[:, :nco], invB[:, :nco], ACTF.Copy)
                for kb in range(KB):
                    nc.vector.tensor_mul(wh[:, kb, c0:c0 + nco],
                                         wstage[:, kb, :nco],
                                         invS[:, :nco])
                return (j, c0, nco)

            LOOKAHEAD = 2   # groups of normalize traced ahead of the m-loop
            normed = {}
            norm_done = 0
            for gi, grp in enumerate(groups):
                while norm_done < min(len(groups), gi + 1 + LOOKAHEAD):
                    for j in groups[norm_done]:
                        normed[j] = normalize_chunk(j)
                    norm_done += 1
                views = [normed[j] for j in grp]

                if stage not in ("mm", "full"):
                    continue
                for m in range(2 * MT):
                    stat = embT_sb if m < MT else wsT_sb
                    mm = m % MT
                    is_ws = m >= MT
                    nmm = KB + (nst if is_ws else 0)
                    pss = [ps_main.tile([128, 512], F32, tag="mm",
                                        name=f"mmps{gi}")
                           for gi in range(len(views))]
                    for kb in range(KB):
                        for gi, (j, c0, nco) in enumerate(views):
                            nc.tensor.matmul(
                                pss[gi][:, :nco],
                                stat[:, kb, mm * 128:(mm + 1) * 128],
                                wh[:, kb, c0:c0 + nco],
                                start=(kb == 0),
                                stop=(not is_ws and kb == KB - 1))
                    if is_ws:
                        for st in range(nst):
                            for gi, (j, c0, nco) in enumerate(views):
                                nc.tensor.matmul(
                                    pss[gi][:, :nco],
                                    u_sb[:, st, mm * 128:(mm + 1) * 128],
                                    v_sb[:, st, c0:c0 + nco],
                                    start=False, stop=(st == nst - 1))
                    for gi, (j, c0, nco) in enumerate(views):
                        ps = pss[gi]
                        if not is_ws:
                            if j == 0:
                                mx = nrm_p.tile([128, 1], F32, tag="mx")
                                nc.vector.reduce_max(mx, ps[:, :nco], axis=AX.X)
                                # slack 46: the chunk-0 row max can undershoot
                                # the true row max by ~3 cos units (90 in
                                # logit units) -> shift exps down, stay finite
                                nc.vector.tensor_scalar(
                                    out=bias_t[:, mm:mm + 1], in0=mx,
                                    scalar1=-S_SCALE, scalar2=-46.0,
                                    op0=ALU.mult, op1=ALU.add)
                            es = escr_p.tile([128, 512], BF16, tag="es")
                            nc.scalar.activation(
                                es[:, :nco], ps[:, :nco], ACTF.Exp,
                                bias=bias_t[:, mm:mm + 1], scale=S_SCALE,
                                accum_out=sslots[:, mm, j:j + 1])
                        else:
                            rr = rscr_p.tile([128, 512], BF16, tag="rr")
                            nc.vector._custom_dve(
                                _SRA_OP, out=rr[:, :nco], in0=ps[:, :nco],
                                s0=nrm_ws[:, mm:mm + 1], s1=_SRA_SEED,
                                imm2=2.0,
                                accum_out=islots[:, mm, j:j + 1])

            if stage != "full":
                probe = {
                    "prep": tgt_t[0:1, 0:1],
                    "norm": wh[0:1, 0, 0:1],
                    "mm": islots[0:1, 0, 0:1],
                }[stage]
                scpy = mrg_p.tile([1, 1], F32)
                nc.vector.tensor_copy(out=scpy, in_=probe)
                nc.sync.dma_start(out=out_p[:, :], in_=scpy)
            else:
                # ---- merge: pack (bias, s, inter) and AllGather ----
                s_t = mrg_p.tile([128, MT], F32)
                for m in range(MT):
                    nc.vector.reduce_sum(s_t[:, m:m + 1], sslots[:, m, :],
                                         axis=AX.X)
                # inter partial: per-m row-sum, scaled by -nrm_i/2 (SRA
                # factoring), then summed over m
                itmp = mrg_p.tile([128, MT], F32)
                for m in range(MT):
                    isum = mrg_p.tile([128, 1], F32, tag=f"isum{m}")
                    nc.vector.reduce_sum(isum, islots[:, m, :], axis=AX.X)
                    nc.vector.tensor_mul(itmp[:, m:m + 1], isum,
                                         scl_ws[:, m:m + 1])
                ipart = mrg_p.tile([128, 1], F32)
                nc.vector.reduce_sum(ipart, itmp, axis=AX.X)

                pack = mrg_p.tile([128, 9], F32)
                nc.vector.tensor_copy(out=pack[:, 0:MT], in_=bias_t)
                nc.vector.tensor_copy(out=pack[:, MT:2 * MT], in_=s_t)
                nc.vector.tensor_copy(out=pack[:, 8:9], in_=ipart)
                nc.sync.dma_start(out=cc_in[:, :], in_=pack[:, :])
                nc.gpsimd.collective_compute(
                    "AllGather", ALU.bypass,
                    replica_groups=[list(range(NCORES))],
                    ins=[cc_in[:, :]], outs=[cc_out[:, :, :]])
                gath = mrg_p.tile([128, NCORES, 9], F32)
                src = cc_out[:, :, :]
                perm = bass.AP(tensor=src.tensor, offset=src.offset,
                               ap=[[9, 128], [128 * 9, NCORES], [1, 9]])
                nc.sync.dma_start(out=gath[:, :, :], in_=perm)

                # ---- final reduction (replicated, batched over the 4 m's) ----
                # (p, core, field) -> (p, field-m, core) permuted views
                bias_v = gath[:, :, 0:MT].rearrange("p c f -> p f c")
                s_v = gath[:, :, MT:2 * MT].rearrange("p c f -> p f c")
                bmin4 = mrg_p.tile([128, MT], F32)
                nc.vector.tensor_reduce(bmin4, bias_v, axis=AX.X, op=ALU.min)
                e48 = mrg_p.tile([128, MT, NCORES], F32)
                for m in range(MT):
                    nc.scalar.activation(e48[:, m, :], bias_v[:, m, :],
                                         ACTF.Exp, bias=bmin4[:, m:m + 1],
                                         scale=-1.0)
                sw = mrg_p.tile([128, MT, NCORES], F32)
                nc.vector.tensor_mul(sw, e48, s_v)
                ssum4 = mrg_p.tile([128, MT], F32)
                nc.vector.reduce_sum(ssum4, sw, axis=AX.X)
                lnv4 = mrg_p.tile([128, MT], F32)
                nc.scalar.activation(lnv4, ssum4, ACTF.Ln)
                cel = mrg_p.tile([128, MT], F32)
                nc.vector.tensor_sub(cel, lnv4, bmin4)           # lse
                nc.vector.tensor_sub(cel, cel, tgt_t)

                fin = mrg_p.tile([128, 2], F32)
                nc.vector.reduce_sum(fin[:, 0:1], cel, axis=AX.X)
                nc.vector.reduce_sum(fin[:, 1:2], gath[:, :, 8], axis=AX.X)
                red = mrg_p.tile([128, 2], F32)
                nc.gpsimd.partition_all_reduce(red, fin, channels=128,
                                               reduce_op=bass_isa.ReduceOp.add)
                ta = mrg_p.tile([1, 1], F32)
                nc.vector.tensor_scalar_mul(ta, red[0:1, 0:1], 1.0 / B)
                tb = mrg_p.tile([1, 1], F32)
                nc.vector.tensor_scalar_mul(tb, red[0:1, 1:2],
                                            LMD / (B * (C - 1.0)))
                res = mrg_p.tile([1, 1], F32)
                nc.vector.tensor_add(res, ta, tb)
                nc.sync.dma_start(out=out_p[:, :], in_=res[:, :])

    nc.compile()
    return nc


@lru_cache(maxsize=4)
def _graph_cached(nst: int, stage: str = "full"):
    return _build_graph(nst, stage)


def _host_shard(emb, W, y):
    emb = np.ascontiguousarray(np.asarray(emb), dtype=np.float32)
    W = np.ascontiguousarray(np.asarray(W), dtype=np.float32)
    y = np.asarray(y).astype(np.int64)

    embT = emb.T.astype(ml_dtypes.bfloat16)
    ws = W[y]                                        # (B, D) f32
    wsT = np.ascontiguousarray(ws.T).astype(ml_dtypes.bfloat16)
    embT = np.ascontiguousarray(embT)

    owner = y // CSH
    local = y % CSH

    slot_lists = []
    for c in range(NCORES):
        rows = np.where(owner == c)[0]
        slot_lists.append(np.unique(local[rows]))
    max_slots = max(len(s) for s in slot_lists)
    nst = max(1, math.ceil(max_slots / 128))

    in_maps = []
    for c in range(NCORES):
        wt_c = np.zeros((D, CPAD), dtype=ml_dtypes.bfloat16)
        wt_c[:, :CSH] = W[c * CSH:(c + 1) * CSH].T.astype(ml_dtypes.bfloat16)
        cls = slot_lists[c]
        U = np.zeros((nst * 128, B), dtype=np.float32)
        V = np.zeros((nst * 128, CPAD), dtype=np.float32)
        if len(cls):
            V[np.arange(len(cls)), cls] = 1.0
            rows = np.where(owner == c)[0]
            slot_of = np.searchsorted(cls, local[rows])
            U[slot_of, rows] = -BIG
        in_maps.append({
            "wt": wt_c,
            "embt": embT,
            "wst": wsT,
            "emb": emb,
            "ws": np.ascontiguousarray(ws),
            "u": U.astype(ml_dtypes.bfloat16),
            "v": V.astype(ml_dtypes.bfloat16),
        })
    return in_maps, nst


def run(emb, W, y, trace=False):
    in_maps, nst = _host_shard(emb, W, y)
    nc = _graph_cached(nst)
    res = run_bass_kernel_spmd(nc, in_maps, core_ids=list(range(NCORES)),
                               trace=trace)
    val = np.float32(res.results[0]["out"][0, 0])
    return val, res


def kernel(emb, W, y):
    val, _ = run(emb, W, y, trace=False)
    return val


if __name__ == "__main__":
    rng = np.random.default_rng(0)
    emb = rng.standard_normal((B, D)).astype(np.float32)
    W = rng.standard_normal((C, D)).astype(np.float32)
    y = rng.integers(0, C, size=(B,)).astype(np.int64)
    print("loss:", kernel(emb, W, y))


# revision 57
# speedup vs baseline: 1.0451x; 1.0451x over previous
"""Distributed Trainium2 kernel for AM-normfree-softmax + MHE inter-class loss.

loss = CE(S*(emb @ normalize(W).T - M*onehot(y)), y)
       + sum_{i, j != y_i} 1/||w_hat_{y_i} - w_hat_j||^2 / (B*(C-1))

Strategy (classifier/tensor parallel): shard the class dim C across 8 cores.
Each core holds its W-shard TRANSPOSED (D, C/8 -> padded 6272) in bf16 as the
moving matmul operand, normalizes it on-device (ACT square -> PE ones-matmul
column sum-of-squares -> rsqrt as exp(-0.5*ln(x)), keeping ACT on one table
set -> PE row-broadcast -> DVE bf16 multiply), and computes
  [emb; W[y]] @ W_hat_shard.T          (stationary = emb.T | W[y].T, bf16)
streamed in 512-col N-chunks (paired per stationary load to hide LDWEIGHTS)
with an extra rank-<=128 K-tile (U @ V, built on the host from y) that
subtracts 2^24 at (i, y_i) so the MHE diagonal self-distance is suppressed
inside the matmul itself - no masking pass, no catastrophic 1/eps terms.

CE epilogue: per-core row-max from the first chunk (+46 slack) fixes a
per-row exp bias; one ACT Exp with accum_out reduces each PSUM tile on the
fly.  MHE epilogue: one custom fused DVE op per tile (SHIFTED_RECIP_ACC_ANT,
registered at import) computing accum += sum_j approx(1/(g_ij - nrm_i)) via
a BITWISE_NOT exponent-flip seed + one Newton pass; the identity
  sum_j 1/(2-2*inv_i*g_ij) = (-nrm_i/2) * sum_j 1/(g_ij - nrm_i)
moves the per-row scale to a single merge-time multiply.

One 4.6KB AllGather of (exp-bias, exp-sum, inter-partial) merges across
cores; each core redundantly computes the final scalar (two-level logsumexp
over 8 core-partials, mean-CE, inter sum).  Only core 0's (1,1) is returned.
"""

import math
from functools import lru_cache

import ml_dtypes
import numpy as np

import concourse.bass as bass
import concourse.bass_isa as bass_isa
import concourse.tile as tile
from concourse import bacc, mybir
from concourse.bass_utils import run_bass_kernel_spmd

# Pin ACT to the one table set containing every function we use
# (exp, ln, copy, identity, square).  The default per-function set picker
# alternates exp_and_others <-> natural_log, reloading tables (~1.5us + drain)
# dozens of times per kernel.  Emptying the other sets' overlapping entries
# (list order and hence act_func_set_id stay intact) forces a single load.
_ACT_KEEP = "natural_log_exp_and_others"
_orig_gat = bacc.get_activation_tables


def _pinned_gat(arch):
    tabs = _orig_gat(arch)
    shared = tabs[_ACT_KEEP]
    return {name: (fns if name == _ACT_KEEP else fns - shared)
            for name, fns in tabs.items()}


bacc.get_activation_tables = _pinned_gat

# ---- custom fused DVE op: accum += sum_k approx(1/(x_k - s0)) ----
# Collapses the MHE epilogue (affine + reciprocal + row-reduce, one ACT op +
# two DVE ops per tile) into a single DVE instruction:
#   sum_j 1/(2 - 2*inv_i*g_ij) == (-nrm_i/2) * sum_j 1/(g_ij - nrm_i)
# so per tile we only need the shifted reciprocal-sum; the (-nrm_i/2) row
# scale is applied once per row at merge time.  BITWISE_NOT exponent-flip
# seed + one Newton pass (naive -4/17 seed consts): ~0.3% systematic error
# on 1/d, i.e. ~1e-5 of the total loss.  6 ALU slices + accumulate.
from operator import add as _op_add  # noqa: E402

import concourse.dve_ops as _dve_ops  # noqa: E402
from concourse.dve_spec import (  # noqa: E402
    AluOp as _DAluOp,
    Bin as _DBin,
    C0 as _DC0,
    C1 as _DC1,
    C2 as _DC2,
    Spec as _DSpec,
    Src0 as _DSrc0,
    Zero as _DZero,
    _has_src1 as _dve_has_src1,
    lower as _dve_lower,
)
from concourse.dve_uop import DveOpSpec as _DveOpSpec  # noqa: E402

_SRA_NAME = "SHIFTED_RECIP_ACC_ANT"


def _sra_reference(in0, in1, s0, s1, imm2):
    x = np.ascontiguousarray(in0.astype(np.float32) - s0)
    nx = (~x.view(np.int32)).view(np.float32)
    y0 = nx * np.float32(s1)
    y1 = (y0 * (np.float32(imm2) - x * y0)).astype(np.float32)
    return y1, y1.reshape(y1.shape[0], -1).sum(axis=-1, keepdims=True)


def _register_sra():
    for op in _dve_ops.OPS:
        if op.name == _SRA_NAME:
            return op
    d = _DSrc0 - _DC0
    nd = _DBin(_DAluOp.BITWISE_NOT, d, d)
    y0 = nd * _DC1
    y1 = y0 * (_DC2 - d * y0)
    spec = _DSpec(body=y1, accum=_op_add, accum_init=_DZero,
                  reference=_sra_reference)
    row = max(_dve_ops._SUB_OPCODE_FOR_NAME.values()) + 1
    assert row < 0x20
    _dve_ops._SUB_OPCODE_FOR_NAME[_SRA_NAME] = row
    shas = {}
    for ver in ("v3", "v4"):
        tmp = _DveOpSpec(name=_SRA_NAME, opcode=row,
                         uops=_dve_lower(spec, ver=ver),
                         rd1_en=_dve_has_src1(spec))
        shas[ver] = tmp.sha(ver)
    op = _dve_ops.DveOp(_SRA_NAME, spec, subdim=False, uops_sha=shas)
    _dve_ops.OPS.append(op)
    _dve_ops.CUSTOM_DVE_SPECS[_SRA_NAME] = spec
    return op


_SRA_OP = _register_sra()
_SRA_SEED = -4.0 / 17.0

F32 = mybir.dt.float32
BF16 = mybir.dt.bfloat16
AX = mybir.AxisListType
ALU = mybir.AluOpType
ACTF = mybir.ActivationFunctionType

B, D, C = 512, 512, 50000
NCORES = 8
CSH = C // NCORES          # 6250 classes per core
CPAD = 6272                # 49 * 128, padded shard width
S_SCALE = 30.0
MARGIN = 0.2
LMD = 1.0
BIG = float(2 ** 24)

# N-chunks over the padded shard: the small 128-col chunk first (cheapest
# prologue: its DMA+normalize chain gates the first matmul), then 12 x 512
CHUNKS = [(6144, 128)] + [(j * 512, 512) for j in range(12)]
NCHUNK = len(CHUNKS)
KB = D // 128              # 4 contraction blocks
MT = B // 128              # 4 M-tiles per operand group (emb rows / ws rows)


def _build_graph(nst: int, stage: str = "full"):
    """Build the SPMD Bass graph. nst = number of 128-slot U/V K-tiles.

    stage: debug knob — "prep", "norm", "mm", or "full".
    """
    nc = bacc.Bacc("TRN2", target_bir_lowering=False, debug=False,
                   num_devices=NCORES)

    wt = nc.declare_dram_parameter("wt", [D, CPAD], BF16, isOutput=False)
    embT = nc.declare_dram_parameter("embt", [D, B], BF16, isOutput=False)
    wsT = nc.declare_dram_parameter("wst", [D, B], BF16, isOutput=False)
    emb = nc.declare_dram_parameter("emb", [B, D], F32, isOutput=False)
    ws = nc.declare_dram_parameter("ws", [B, D], F32, isOutput=False)
    u_p = nc.declare_dram_parameter("u", [nst * 128, B], BF16, isOutput=False)
    v_p = nc.declare_dram_parameter("v", [nst * 128, CPAD], BF16, isOutput=False)
    out_p = nc.declare_dram_parameter("out", [1, 1], F32, isOutput=True)

    cc_in = nc.dram_tensor("cc_in", [128, 9], F32)
    cc_out = nc.dram_tensor("cc_out", [NCORES, 128, 9], F32, addr_space="Shared")

    with tile.TileContext(nc) as tc:
        with (
            tc.tile_pool(name="consts", bufs=1) as consts,
            tc.tile_pool(name="stat", bufs=1) as statp,
            tc.tile_pool(name="persist", bufs=1) as pers,
            tc.tile_pool(name="wstage", bufs=6) as wstage_p,
            tc.tile_pool(name="sq", bufs=6) as sq_p,
            tc.tile_pool(name="nrm", bufs=2) as nrm_p,
            tc.tile_pool(name="escr", bufs=3) as escr_p,
            tc.tile_pool(name="rscr", bufs=3) as rscr_p,
            tc.tile_pool(name="mrg", bufs=1) as mrg_p,
            tc.tile_pool(name="ps_main", bufs=4, space="PSUM") as ps_main,
            tc.tile_pool(name="ps_ssq", bufs=2, space="PSUM") as ps_ssq,
            tc.tile_pool(name="ps_inv", bufs=2, space="PSUM") as ps_inv,
        ):
            # ---- constants ----
            ones_col = consts.tile([128, 1], BF16)     # sumsq lhsT (K=128,M=1)
            nc.vector.memset(ones_col, 1.0)
            ones_row = consts.tile([1, 128], BF16)     # bcast lhsT (K=1,M=128)
            nc.vector.memset(ones_row, 1.0)
            eps_t = consts.tile([1, 1], F32)           # sqrt bias for pad cols
            nc.vector.memset(eps_t, 1e-30)
            # dummy activation traced first: pulls the one-time ACT table load
            # (~2.7us) off the first chunk's critical path
            warm_t = consts.tile([1, 1], F32)
            nc.scalar.activation(warm_t, eps_t, ACTF.Square)

            # ---- chunk-0 W DMA + square first: shortens the prologue (the
            # first matmuls depend on this chain, not on the stationaries)
            wst0 = None
            if stage != "prep":
                c0_0, nco_0 = CHUNKS[0]
                wst0 = wstage_p.tile([128, KB, 512], BF16, tag="wstage")
                sq0 = sq_p.tile([128, KB, 512], BF16, tag="sq")
                for kb in range(KB):
                    nc.sync.dma_start(
                        out=wst0[:, kb, :nco_0],
                        in_=wt[kb * 128:(kb + 1) * 128, c0_0:c0_0 + nco_0])
                    nc.scalar.activation(sq0[:, kb, :nco_0],
                                         wst0[:, kb, :nco_0], ACTF.Square)

            # ---- stationary operands (embT/wsT on the sync queue right after
            # chunk 0; bulky V/emb/ws on the gpsimd queue in parallel) ----
            embT_sb = statp.tile([128, KB, B], BF16)
            wsT_sb = statp.tile([128, KB, B], BF16)
            for kb in range(KB):
                nc.sync.dma_start(out=embT_sb[:, kb, :],
                                  in_=embT[kb * 128:(kb + 1) * 128, :])
                nc.sync.dma_start(out=wsT_sb[:, kb, :],
                                  in_=wsT[kb * 128:(kb + 1) * 128, :])
            u_sb = statp.tile([128, nst, B], BF16)
            for st in range(nst):
                nc.gpsimd.dma_start(out=u_sb[:, st, :],
                                    in_=u_p[st * 128:(st + 1) * 128, :])
            v_sb = statp.tile([128, nst, CPAD], BF16)
            for st in range(nst):
                nc.gpsimd.dma_start(out=v_sb[:, st, :],
                                    in_=v_p[st * 128:(st + 1) * 128, :])

            # natural-layout emb/ws for target-logit extraction
            emb_sb = statp.tile([128, MT, D], F32)
            ws_sb = statp.tile([128, MT, D], F32)
            for m in range(MT):
                nc.gpsimd.dma_start(out=emb_sb[:, m, :],
                                    in_=emb[m * 128:(m + 1) * 128, :])
                nc.gpsimd.dma_start(out=ws_sb[:, m, :],
                                    in_=ws[m * 128:(m + 1) * 128, :])

            # ---- ws row norms + target logits (replicated on all cores) ----
            ssq_ws = pers.tile([128, MT], F32)
            dot_t = pers.tile([128, MT], F32)
            ttr_scr = pers.tile([128, D], F32)
            for m in range(MT):
                nc.vector.tensor_mul(ttr_scr, ws_sb[:, m, :], ws_sb[:, m, :])
                nc.vector.reduce_sum(ssq_ws[:, m:m + 1], ttr_scr, axis=AX.X)
            for m in range(MT):
                nc.vector.tensor_mul(ttr_scr, emb_sb[:, m, :], ws_sb[:, m, :])
                nc.vector.reduce_sum(dot_t[:, m:m + 1], ttr_scr, axis=AX.X)
            # inv_ws = rsqrt(ssq) = exp(-0.5*ln(ssq)); Ln/Exp share one ACT
            # table set (no per-use table reloads, no slow DVE reciprocal)
            lnv_ws = pers.tile([128, MT], F32)
            nc.scalar.activation(lnv_ws, ssq_ws, ACTF.Ln)
            inv_ws = pers.tile([128, MT], F32)
            nc.scalar.activation(inv_ws, lnv_ws, ACTF.Exp, scale=-0.5)
            nrm_ws = pers.tile([128, MT], F32)     # ||w_{y_i}||, SRA shift
            nc.scalar.activation(nrm_ws, lnv_ws, ACTF.Exp, scale=0.5)
            scl_ws = pers.tile([128, MT], F32)     # -nrm/2, SRA row scale
            nc.vector.tensor_scalar_mul(scl_ws, nrm_ws, -0.5)
            # tgt = S*(inv_ws*dot - MARGIN)
            cosiy = pers.tile([128, MT], F32)
            nc.vector.tensor_mul(cosiy, dot_t, inv_ws)
            tgt_t = pers.tile([128, MT], F32)
            nc.vector.tensor_scalar(out=tgt_t, in0=cosiy,
                                    scalar1=S_SCALE, scalar2=-S_SCALE * MARGIN,
                                    op0=ALU.mult, op1=ALU.add)

            # ---- persistent accumulators ----
            wh = pers.tile([128, KB, CPAD], BF16)       # normalized W-shard.T
            bias_t = pers.tile([128, MT], F32)          # per-row exp bias
            sslots = pers.tile([128, MT, NCHUNK], F32)  # per-chunk exp sums
            islots = pers.tile([128, MT, NCHUNK], F32)  # per-chunk 1/d2 sums

            if stage == "prep":
                # touch wt so the ExternalInput isn't pruned
                wtouch = wstage_p.tile([128, 512], BF16, tag="wtouch")
                nc.sync.dma_start(out=wtouch, in_=wt[0:128, 0:512])

            # ---- main loop over N-chunks ----
            chunk_list = [] if stage == "prep" else (
                CHUNKS if stage in ("mm", "full") else CHUNKS[:2])
            # pair the 512-col chunks so each stationary load covers two
            # moving streams (halves exposed LDWEIGHTS); chunk 0 (128 cols)
            # runs alone to keep the prologue short
            groups, idx = [], 0
            while idx < len(chunk_list):
                n = 1 if idx < 4 else min(2, len(chunk_list) - idx)
                groups.append(list(range(idx, idx + n)))
                idx += n

            def normalize_chunk(j):
                c0, nco = chunk_list[j]
                if j == 0 and wst0 is not None:
                    wstage, sq = wst0, sq0
                else:
                    wstage = wstage_p.tile([128, KB, 512], BF16, tag="wstage",
                                           name=f"wstage{j}")
                    sq = sq_p.tile([128, KB, 512], BF16, tag="sq",
                                   name=f"sq{j}")
                    for kb in range(KB):
                        nc.sync.dma_start(
                            out=wstage[:, kb, :nco],
                            in_=wt[kb * 128:(kb + 1) * 128, c0:c0 + nco])
                        nc.scalar.activation(sq[:, kb, :nco],
                                             wstage[:, kb, :nco],
                                             ACTF.Square)
                ssq_ps = ps_ssq.tile([1, 512], F32, tag="ssq", name=f"ssq{j}")
                for kb in range(KB):
                    nc.tensor.matmul(ssq_ps[:, :nco], ones_col,
                                     sq[:, kb, :nco],
                                     start=(kb == 0), stop=(kb == KB - 1))
                # inv = rsqrt(ssq + eps) = exp(-0.5*ln(ssq + eps))
                nrm = nrm_p.tile([1, 512], F32, tag="nrm", name=f"nrm{j}")
                nc.scalar.activation(nrm[:, :nco], ssq_ps[:, :nco],
                                     ACTF.Ln, bias=eps_t[:, :])
                inv_bf = nrm_p.tile([1, 512], BF16, tag="invbf",
                                    name=f"invbf{j}")
                nc.scalar.activation(inv_bf[:, :nco], nrm[:, :nco],
                                     ACTF.Exp, scale=-0.5)
                invB = ps_inv.tile([128, 512], F32, tag="invB",
                                   name=f"invB{j}")
                nc.tensor.matmul(invB[:, :nco], ones_row, inv_bf[:, :nco],
                                 start=True, stop=True)
                # PSUM->SBUF bf16 copy on ACT so the normalize multiply runs
                # bf16 x bf16 at the DVE 2x rate
                invS = nrm_p.tile([128, 512], BF16, tag="invS",
                                  name=f"invS{j}")
                nc.scalar.activation(invS[:, :nco], invB[:, :nco], ACTF.Copy)
                for kb in range(KB):
                    nc.vector.tensor_mul(wh[:, kb, c0:c0 + nco],
                                         wstage[:, kb, :nco],
                                         invS[:, :nco])
                return (j, c0, nco)

            LOOKAHEAD = 2   # groups of normalize traced ahead of the m-loop
            normed = {}
            norm_done = 0
            for gi, grp in enumerate(groups):
                while norm_done < min(len(groups), gi + 1 + LOOKAHEAD):
                    for j in groups[norm_done]:
                        normed[j] = normalize_chunk(j)
                    norm_done += 1
                views = [normed[j] for j in grp]

                if stage not in ("mm", "full"):
                    continue
                for m in range(2 * MT):
                    stat = embT_sb if m < MT else wsT_sb
                    mm = m % MT
                    is_ws = m >= MT
                    nmm = KB + (nst if is_ws else 0)
                    pss = [ps_main.tile([128, 512], F32, tag="mm",
                                        name=f"mmps{gi}")
                           for gi in range(len(views))]
                    for kb in range(KB):
                        for gi, (j, c0, nco) in enumerate(views):
                            nc.tensor.matmul(
                                pss[gi][:, :nco],
                                stat[:, kb, mm * 128:(mm + 1) * 128],
                                wh[:, kb, c0:c0 + nco],
                                start=(kb == 0),
                                stop=(not is_ws and kb == KB - 1))
                    if is_ws:
                        for st in range(nst):
                            for gi, (j, c0, nco) in enumerate(views):
                                nc.tensor.matmul(
                                    pss[gi][:, :nco],
                                    u_sb[:, st, mm * 128:(mm + 1) * 128],
                                    v_sb[:, st, c0:c0 + nco],
                                    start=False, stop=(st == nst - 1))
                    for gi, (j, c0, nco) in enumerate(views):
                        ps = pss[gi]
                        if not is_ws:
                            if j == 0:
                                mx = nrm_p.tile([128, 1], F32, tag="mx")
                                nc.vector.reduce_max(mx, ps[:, :nco], axis=AX.X)
                                # slack 46: the chunk-0 row max can undershoot
                                # the true row max by ~3 cos units (90 in
                                # logit units) -> shift exps down, stay finite
                                nc.vector.tensor_scalar(
                                    out=bias_t[:, mm:mm + 1], in0=mx,
                                    scalar1=-S_SCALE, scalar2=-46.0,
                                    op0=ALU.mult, op1=ALU.add)
                            es = escr_p.tile([128, 512], BF16, tag="es")
                            nc.scalar.activation(
                                es[:, :nco], ps[:, :nco], ACTF.Exp,
                                bias=bias_t[:, mm:mm + 1], scale=S_SCALE,
                                accum_out=sslots[:, mm, j:j + 1])
                        else:
                            rr = rscr_p.tile([128, 512], BF16, tag="rr")
                            nc.vector._custom_dve(
                                _SRA_OP, out=rr[:, :nco], in0=ps[:, :nco],
                                s0=nrm_ws[:, mm:mm + 1], s1=_SRA_SEED,
                                imm2=2.0,
                                accum_out=islots[:, mm, j:j + 1])

            if stage != "full":
                probe = {
                    "prep": tgt_t[0:1, 0:1],
                    "norm": wh[0:1, 0, 0:1],
                    "mm": islots[0:1, 0, 0:1],
                }[stage]
                scpy = mrg_p.tile([1, 1], F32)
                nc.vector.tensor_copy(out=scpy, in_=probe)
                nc.sync.dma_start(out=out_p[:, :], in_=scpy)
            else:
                # ---- merge: pack (bias, s, inter) and AllGather ----
                s_t = mrg_p.tile([128, MT], F32)
                for m in range(MT):
                    nc.vector.reduce_sum(s_t[:, m:m + 1], sslots[:, m, :],
                                         axis=AX.X)
                # inter partial: per-m row-sum, scaled by -nrm_i/2 (SRA
                # factoring), then summed over m
                itmp = mrg_p.tile([128, MT], F32)
                for m in range(MT):
                    isum = mrg_p.tile([128, 1], F32, tag=f"isum{m}")
                    nc.vector.reduce_sum(isum, islots[:, m, :], axis=AX.X)
                    nc.vector.tensor_mul(itmp[:, m:m + 1], isum,
                                         scl_ws[:, m:m + 1])
                ipart = mrg_p.tile([128, 1], F32)
                nc.vector.reduce_sum(ipart, itmp, axis=AX.X)

                pack = mrg_p.tile([128, 9], F32)
                nc.vector.tensor_copy(out=pack[:, 0:MT], in_=bias_t)
                nc.vector.tensor_copy(out=pack[:, MT:2 * MT], in_=s_t)
                nc.vector.tensor_copy(out=pack[:, 8:9], in_=ipart)
                nc.sync.dma_start(out=cc_in[:, :], in_=pack[:, :])
                nc.gpsimd.collective_compute(
                    "AllGather", ALU.bypass,
                    replica_groups=[list(range(NCORES))],
                    ins=[cc_in[:, :]], outs=[cc_out[:, :, :]])
                gath = mrg_p.tile([128, NCORES, 9], F32)
                src = cc_out[:, :, :]
                perm = bass.AP(tensor=src.tensor, offset=src.offset,
                               ap=[[9, 128], [128 * 9, NCORES], [1, 9]])
                nc.sync.dma_start(out=gath[:, :, :], in_=perm)

                # ---- final reduction (replicated, batched over the 4 m's) ----
                # (p, core, field) -> (p, field-m, core) permuted views
                bias_v = gath[:, :, 0:MT].rearrange("p c f -> p f c")
                s_v = gath[:, :, MT:2 * MT].rearrange("p c f -> p f c")
                bmin4 = mrg_p.tile([128, MT], F32)
                nc.vector.tensor_reduce(bmin4, bias_v, axis=AX.X, op=ALU.min)
                e48 = mrg_p.tile([128, MT, NCORES], F32)
                for m in range(MT):
                    nc.scalar.activation(e48[:, m, :], bias_v[:, m, :],
                                         ACTF.Exp, bias=bmin4[:, m:m + 1],
                                         scale=-1.0)
                sw = mrg_p.tile([128, MT, NCORES], F32)
                nc.vector.tensor_mul(sw, e48, s_v)
                ssum4 = mrg_p.tile([128, MT], F32)
                nc.vector.reduce_sum(ssum4, sw, axis=AX.X)
                lnv4 = mrg_p.tile([128, MT], F32)
                nc.scalar.activation(lnv4, ssum4, ACTF.Ln)
                cel = mrg_p.tile([128, MT], F32)
                nc.vector.tensor_sub(cel, lnv4, bmin4)           # lse
                nc.vector.tensor_sub(cel, cel, tgt_t)

                fin = mrg_p.tile([128, 2], F32)
                nc.vector.reduce_sum(fin[:, 0:1], cel, axis=AX.X)
                nc.vector.reduce_sum(fin[:, 1:2], gath[:, :, 8], axis=AX.X)
                red = mrg_p.tile([128, 2], F32)
                nc.gpsimd.partition_all_reduce(red, fin, channels=128,
                                               reduce_op=bass_isa.ReduceOp.add)
                ta = mrg_p.tile([1, 1], F32)
                nc.vector.tensor_scalar_mul(ta, red[0:1, 0:1], 1.0 / B)
                tb = mrg_p.tile([1, 1], F32)
                nc.vector.tensor_scalar_mul(tb, red[0:1, 1:2],
                                            LMD / (B * (C - 1.0)))
                res = mrg_p.tile([1, 1], F32)
                nc.vector.tensor_add(res, ta, tb)
                nc.sync.dma_start(out=out_p[:, :], in_=res[:, :])

    nc.compile()
    return nc


@lru_cache(maxsize=4)
def _graph_cached(nst: int, stage: str = "full"):
    return _build_graph(nst, stage)


def _host_shard(emb, W, y):
    emb = np.ascontiguousarray(np.asarray(emb), dtype=np.float32)
    W = np.ascontiguousarray(np.asarray(W), dtype=np.float32)
    y = np.asarray(y).astype(np.int64)

    embT = emb.T.astype(ml_dtypes.bfloat16)
    ws = W[y]                                        # (B, D) f32
    wsT = np.ascontiguousarray(ws.T).astype(ml_dtypes.bfloat16)
    embT = np.ascontiguousarray(embT)

    owner = y // CSH
    local = y % CSH

    slot_lists = []
    for c in range(NCORES):
        rows = np.where(owner == c)[0]
        slot_lists.append(np.unique(local[rows]))
    max_slots = max(len(s) for s in slot_lists)
    nst = max(1, math.ceil(max_slots / 128))

    in_maps = []
    for c in range(NCORES):
        wt_c = np.zeros((D, CPAD), dtype=ml_dtypes.bfloat16)
        wt_c[:, :CSH] = W[c * CSH:(c + 1) * CSH].T.astype(ml_dtypes.bfloat16)
        cls = slot_lists[c]
        U = np.zeros((nst * 128, B), dtype=np.float32)
        V = np.zeros((nst * 128, CPAD), dtype=np.float32)
        if len(cls):
            V[np.arange(len(cls)), cls] = 1.0
            rows = np.where(owner == c)[0]
            slot_of = np.searchsorted(cls, local[rows])
            U[slot_of, rows] = -BIG
        in_maps.append({
            "wt": wt_c,
            "embt": embT,
            "wst": wsT,
            "emb": emb,
            "ws": np.ascontiguousarray(ws),
            "u": U.astype(ml_dtypes.bfloat16),
            "v": V.astype(ml_dtypes.bfloat16),
        })
    return in_maps, nst


def run(emb, W, y, trace=False):
    in_maps, nst = _host_shard(emb, W, y)
    nc = _graph_cached(nst)
    res = run_bass_kernel_spmd(nc, in_maps, core_ids=list(range(NCORES)),
                               trace=trace)
    val = np.float32(res.results[0]["out"][0, 0])
    return val, res


def kernel(emb, W, y):
    val, _ = run(emb, W, y, trace=False)
    return val


if __name__ == "__main__":
    rng = np.random.default_rng(0)
    emb = rng.standard_normal((B, D)).astype(np.float32)
    W = rng.standard_normal((C, D)).astype(np.float32)
    y = rng.integers(0, C, size=(B,)).astype(np.int64)
    print("loss:", kernel(emb, W, y))
